# revision 15
# baseline (speedup 1.0000x reference)
"""GCN-style 8-step SpMM power iteration on 8 Trainium2 NeuronCores.

Math (reference):
    deg = segment_sum(1, col); dis = rsqrt(max(deg,1)) where deg>0 else 0
    norm_e = dis[row_e] * dis[col_e];  row' = row - row.min()
    xX = x @ W_linX + b_linX
    hX_{t+1}[v] = sum_{e: row'_e = v} norm_e * hX_t[col_e] + xX[v]   (8 times)
    out = relu(pp0*xX + pp1*hX_8) @ W_pred + b_pred

Key algebraic trick: norm factorizes per-edge into src/dst node factors, so we
keep the node table pre-scaled: T = dis ⊙ hX. Then one step is
    S[v]   = sum_{e->v} T[col_e]            (pure gather + segment-sum, no
                                             per-edge arithmetic at all)
    T'[v]  = (dis*dis_sh)[v]*S[v] + (dis*xX)[v]   (one DVE op per group)

Distribution: nodes dst-sharded over 8 cores. Each core owns a contiguous
slice of a relabeled "slot" table; per-iteration AllGather rebuilds the full
table on every core. The AllGather is split in two halves (slots L/H per
core) so the first half can be exchanged while the second half still
computes. Gather of source rows uses dma_gather (int16 indices -> table
split in L/H halves; edges partitioned by source slot-half). The gathers are
spread across all 4 SWDGE queues (each queue is served by its own pair of
GPSIMD Q7 cores -> 4x descriptor-generation throughput; descriptor
generation is the kernel's bottleneck). Segment-sum runs on the
TensorEngine: edges are binned into sub-blocks of <=32 destination nodes
with a fixed budget of 2 chunks (128 edges each) per source-half; each
chunk's 0/1 selection matrix S (fp16, host-built) is the stationary matmul
operand, the gathered fp16 messages the moving one, accumulating fp32 in
PSUM.
"""

import numpy as np

# problem shape (hardcoded per the task contract)
N = 50000
E = 800000
IN_C = 128
HID = 128
OUT_C = 40
POWER1 = 8

NCORES = 8
SUB_NODES = 32          # destination slots per sub-block (= matmul M)
CHUNK = 128             # edges per chunk (= matmul K)
A_CHUNKS = 2            # chunks per sub-block from source half L
B_CHUNKS = 2
BATCH_GROUPS = 3        # psum groups (of 4 sub-blocks) per gather batch
GROUP_SUBS = 4          # sub-blocks per psum group ([128,128] psum tile)
NQUEUES = 4             # SWDGE queues used for the gathers


# ----------------------------------------------------------------------------
# Host-side preprocessing
# ----------------------------------------------------------------------------

def _pack_core(degA, degB, capA, capB, sub_nodes):
    """2D best-fit-decreasing bin packing of nodes into sub-blocks."""
    order = np.argsort(-np.maximum(degA, degB), kind="stable")
    bins = []        # (node_list, sumA, sumB)
    for v in order:
        a, b = int(degA[v]), int(degB[v])
        best, best_slack = -1, None
        for i, (nodes, sa, sb) in enumerate(bins):
            if len(nodes) < sub_nodes and sa + a <= capA and sb + b <= capB:
                slack = (capA - sa - a) + (capB - sb - b)
                if best_slack is None or slack < best_slack:
                    best, best_slack = i, slack
        if best < 0:
            bins.append(([v], a, b))
        else:
            nodes, sa, sb = bins[best]
            nodes.append(v)
            bins[best] = (nodes, sa + a, sb + b)
    return [b[0] for b in bins]


def _preprocess(inputs, n=N, ncores=NCORES):
    x = np.asarray(inputs["x"], dtype=np.float32)
    edge_index = np.asarray(inputs["edge_index"])
    W_linX = np.asarray(inputs["W_linX"], dtype=np.float32)
    b_linX = np.asarray(inputs["b_linX"], dtype=np.float32)
    policy = np.asarray(inputs["policy"], dtype=np.float64)
    W_pred = np.asarray(inputs["W_pred"], dtype=np.float32)
    b_pred = np.asarray(inputs["b_pred"], dtype=np.float32)

    npc = n // ncores
    half_npc = npc // 2
    row = edge_index[0].astype(np.int64)
    col = edge_index[1].astype(np.int64)
    deg = np.bincount(col, minlength=n).astype(np.float64)
    dis = np.where(deg > 0, 1.0 / np.sqrt(np.maximum(deg, 1.0)), 0.0)
    shift = int(row.min())
    dst = row - shift                      # aggregation destination
    # per-dst factor is dis at the *unshifted* row id
    dis_sh = np.zeros(n, dtype=np.float64)
    hi = n - shift
    dis_sh[:hi] = dis[shift:]

    e = np.exp(policy[:2] - policy[:2].max())
    pp = e / e.sum()
    pp0, pp1 = float(pp[0]), float(pp[1])
    b_comb = pp0 + pp1                      # == 1.0, but don't rely on it

    # a-priori source-half assignment: node v is in half L iff its local id
    # within its core is < npc/2. Needed before packing so per-dst source
    # half degrees are known.
    loc = np.arange(n) % npc
    node_isL = loc < half_npc

    capA, capB = A_CHUNKS * CHUNK, B_CHUNKS * CHUNK

    cores = []
    max_bins_L = max_bins_H = 0
    for c in range(ncores):
        m = (dst >= c * npc) & (dst < (c + 1) * npc)
        e_dst = dst[m] - c * npc
        e_src = col[m]
        srcL = node_isL[e_src]
        degA = np.bincount(e_dst[srcL], minlength=npc)
        degB = np.bincount(e_dst[~srcL], minlength=npc)
        # pack the core's own L-nodes into the first-half bins and H-nodes
        # into the second-half bins (a node's slot half must match its
        # a-priori source-half so consumers know which table to gather from)
        binsL = _pack_core(degA[:half_npc], degB[:half_npc], capA, capB,
                           SUB_NODES)
        binsH = _pack_core(degA[half_npc:], degB[half_npc:], capA, capB,
                           SUB_NODES)
        binsH = [[v + half_npc for v in b] for b in binsH]
        max_bins_L = max(max_bins_L, len(binsL))
        max_bins_H = max(max_bins_H, len(binsH))
        cores.append((e_dst, e_src, binsL, binsH))

    # per-half sub-block count, aligned so psum groups (GROUP_SUBS sub-blocks)
    # do not straddle the half boundary; batches may be ragged.
    n_sub_h = -(-max(max_bins_L, max_bins_H) // GROUP_SUBS) * GROUP_SUBS
    n_sub = 2 * n_sub_h
    slots = n_sub * SUB_NODES               # table rows per core
    hslots = slots // 2                     # rows per half per core
    half_rows = hslots * ncores             # rows of each shared table half
    assert half_rows <= 32767, f"table half {half_rows} exceeds int16 range"
    n_grp = slots // 128

    # slot assignment + per-half global row map
    slot_of_node = np.full(n, -1, dtype=np.int64)   # local slot within core
    for c, (e_dst, e_src, binsL, binsH) in enumerate(cores):
        for bi, nodes in enumerate(binsL):
            for k, v in enumerate(nodes):
                slot_of_node[c * npc + v] = bi * SUB_NODES + k
        for bi, nodes in enumerate(binsH):
            for k, v in enumerate(nodes):
                slot_of_node[c * npc + v] = hslots + bi * SUB_NODES + k
    # global row in the (L|H) shared table
    core_of = np.arange(n) // npc
    lslot = slot_of_node
    is_slotL = (lslot >= 0) & (lslot < hslots)
    grow = np.where(is_slotL, core_of * hslots + lslot,
                    core_of * hslots + (lslot - hslots))

    # an empty slot per half (of core 0) for padding gathers (zero rows)
    padL = padH = -1
    usedL = np.zeros(hslots, dtype=bool)
    usedH = np.zeros(hslots, dtype=bool)
    sl0 = slot_of_node[0:npc]
    usedL[sl0[(sl0 >= 0) & (sl0 < hslots)]] = True
    usedH[sl0[sl0 >= hslots] - hslots] = True
    freeL = np.flatnonzero(~usedL)
    freeH = np.flatnonzero(~usedH)
    assert freeL.size > 0 and freeH.size > 0
    padL, padH = int(freeL[0]), int(freeH[0])

    per_core = []
    nchA = n_sub * A_CHUNKS
    nchB = n_sub * B_CHUNKS
    for c, (e_dst, e_src, binsL, binsH) in enumerate(cores):
        dloc = slot_of_node[e_dst + c * npc]
        dst_bin = np.where(dloc < hslots, dloc // SUB_NODES,
                           n_sub_h + (dloc - hslots) // SUB_NODES)
        dst_k = dloc % SUB_NODES
        isA = node_isL[e_src]
        S = np.zeros((nchA + nchB, CHUNK, SUB_NODES), dtype=np.float16)
        idxA = np.full(nchA * CHUNK, padL, dtype=np.int64)
        idxB = np.full(nchB * CHUNK, padH, dtype=np.int64)
        for bi in range(n_sub):
            for half in (True, False):
                sel = (dst_bin == bi) & (isA == half)
                srcs = e_src[sel]
                dks = dst_k[sel]
                kk = len(srcs)
                cap = capA if half else capB
                assert kk <= cap, (c, bi, half, kk)
                if half:
                    cbase, sbase, idx = bi * A_CHUNKS, 0, idxA
                else:
                    cbase, sbase, idx = bi * B_CHUNKS, nchA, idxB
                for j in range(kk):
                    ch = cbase + j // CHUNK
                    S[sbase + ch, j % CHUNK, dks[j]] = 1.0
                idx[cbase * CHUNK: cbase * CHUNK + kk] = grow[srcs]
        assert idxA.min() >= 0 and idxA.max() < half_rows
        assert idxB.min() >= 0 and idxB.max() < half_rows

        def wrap_idx(idx):
            # index i consumed from [i % 16, i // 16]; replicate to 128 parts
            w = idx.reshape(-1, 16).T.astype(np.int16)      # [16, n/16]
            return np.tile(w, (8, 1))                        # [128, n/16]

        # slot-layout host arrays
        x_slot = np.zeros((slots, IN_C), dtype=np.float32)
        dis_slot = np.zeros(slots, dtype=np.float64)
        dsh_slot = np.zeros(slots, dtype=np.float64)
        nodes_c = np.arange(c * npc, (c + 1) * npc)
        sl = slot_of_node[nodes_c]
        x_slot[sl] = x[nodes_c]
        dis_slot[sl] = dis[nodes_c]
        dsh_slot[sl] = dis_sh[nodes_c]

        grp = lambda v: v.reshape(n_grp, 128).T.astype(np.float32)
        per_core.append({
            "x_slot": x_slot,
            "S": np.ascontiguousarray(
                S.transpose(1, 0, 2).reshape(CHUNK, -1)),    # [128, TC*32]
            "idxA": wrap_idx(idxA),
            "idxB": wrap_idx(idxB),
            "disg": grp(dis_slot),
            "dis2g": grp(dis_slot * dsh_slot),
            "ag": grp((pp1 / b_comb) * dsh_slot),
            "W_linX": W_linX,
            "bX": np.tile(b_linX[None, :], (128, 1)).astype(np.float32),
            "W_pred": (b_comb * W_pred).astype(np.float32),
            "bP": np.tile(b_pred[None, :], (128, 1)).astype(np.float32),
            "ident": np.eye(128, dtype=np.float32),
        })

    meta = dict(n=n, ncores=ncores, npc=npc, n_sub=n_sub, n_sub_h=n_sub_h,
                n_grp=n_grp, slots=slots, hslots=hslots, half_rows=half_rows,
                slot_of_node=slot_of_node)
    return meta, per_core


# ----------------------------------------------------------------------------
# Bass program
# ----------------------------------------------------------------------------

def _build_program(meta, iters=POWER1):
    import concourse.bacc as bacc
    import concourse.mybir as mybir
    from concourse import tile

    f32, f16, i16 = mybir.dt.float32, mybir.dt.float16, mybir.dt.int16
    ADD, MULT = mybir.AluOpType.add, mybir.AluOpType.mult

    ncores = meta["ncores"]
    n_sub, n_grp = meta["n_sub"], meta["n_grp"]
    slots, hslots = meta["slots"], meta["hslots"]
    half_rows = meta["half_rows"]
    nchA = n_sub * A_CHUNKS
    batch_idx = BATCH_GROUPS * GROUP_SUBS * A_CHUNKS * CHUNK
    TC = n_sub * (A_CHUNKS + B_CHUNKS)
    hgrp = n_grp // 2
    # ragged batches: BATCH_GROUPS psum groups each, not straddling halves
    batches = []
    for h in range(2):
        g = h * hgrp
        while g < (h + 1) * hgrp:
            k = min(BATCH_GROUPS, (h + 1) * hgrp - g)
            batches.append((g, k))
            g += k

    nc = bacc.Bacc("TRN2", target_bir_lowering=False, debug=False,
                   enable_asserts=False, num_devices=ncores,
                   num_swdge_queues=NQUEUES,
                   dynamic_dma_scratch_size=32768)

    x_slot_h = nc.dram_tensor("x_slot", [slots, IN_C], f32, kind="ExternalInput")
    S_h = nc.dram_tensor("S", [CHUNK, TC * SUB_NODES], f16, kind="ExternalInput")
    idxA_h = nc.dram_tensor("idxA", [128, nchA * CHUNK // 16], i16,
                            kind="ExternalInput")
    idxB_h = nc.dram_tensor("idxB", [128, n_sub * B_CHUNKS * CHUNK // 16], i16,
                            kind="ExternalInput")
    disg_h = nc.dram_tensor("disg", [128, n_grp], f32, kind="ExternalInput")
    dis2g_h = nc.dram_tensor("dis2g", [128, n_grp], f32, kind="ExternalInput")
    ag_h = nc.dram_tensor("ag", [128, n_grp], f32, kind="ExternalInput")
    W_h = nc.dram_tensor("W_linX", [IN_C, HID], f32, kind="ExternalInput")
    bX_h = nc.dram_tensor("bX", [128, HID], f32, kind="ExternalInput")
    Wp_h = nc.dram_tensor("W_pred", [HID, OUT_C], f32, kind="ExternalInput")
    bP_h = nc.dram_tensor("bP", [128, OUT_C], f32, kind="ExternalInput")
    id_h = nc.dram_tensor("ident", [128, 128], f32, kind="ExternalInput")

    # ping-pong shared tables, split in L/H halves so the L AllGather can
    # launch while H still computes
    tabL = [nc.dram_tensor(f"tabL{p}", [half_rows, HID], f16,
                           addr_space="Shared") for p in range(2)]
    tabH = [nc.dram_tensor(f"tabH{p}", [half_rows, HID], f16,
                           addr_space="Shared") for p in range(2)]
    shardL = [nc.dram_tensor(f"shardL{p}", [hslots, HID], f16)
              for p in range(2)]
    shardH = [nc.dram_tensor(f"shardH{p}", [hslots, HID], f16)
              for p in range(2)]
    out_h = nc.dram_tensor("out", [slots, OUT_C], f32, kind="ExternalOutput")

    rg = [list(range(ncores))]

    def stage_dma(stage, g, parity):
        # route group g's staged rows to the right shard half
        if g < hgrp:
            rows = slice(g * 128, (g + 1) * 128)
            nc.sync.dma_start(shardL[parity][rows, :], stage[:, :])
        else:
            rows = slice((g - hgrp) * 128, (g - hgrp + 1) * 128)
            nc.sync.dma_start(shardH[parity][rows, :], stage[:, :])

    def allgather(src, dst):
        nc.gpsimd.collective_compute(
            "AllGather", mybir.AluOpType.bypass, replica_groups=rg,
            ins=[src.ap().opt()], outs=[dst.ap().opt()])

    with tile.TileContext(nc, num_cores=ncores) as tc:
        import contextlib
        with contextlib.ExitStack() as ctx:
            cpool = ctx.enter_context(tc.tile_pool(name="const", bufs=1))
            wpool = ctx.enter_context(tc.tile_pool(name="work", bufs=2))
            gpool = ctx.enter_context(tc.tile_pool(name="gathA", bufs=6))
            gpoolB = ctx.enter_context(tc.tile_pool(name="gathB", bufs=3))
            spool = ctx.enter_context(tc.tile_pool(name="stage", bufs=3))
            ppool = ctx.enter_context(
                tc.tile_pool(name="psum", bufs=6, space="PSUM"))
            tpool = ctx.enter_context(
                tc.tile_pool(name="psum2", bufs=1, space="PSUM"))

            # persistent SBUF
            S_sb = cpool.tile([CHUNK, TC * SUB_NODES], f16)
            nc.sync.dma_start(S_sb[:, :], S_h[:, :])
            idxA_sb = cpool.tile([128, nchA * CHUNK // 16], i16)
            nc.sync.dma_start(idxA_sb[:, :], idxA_h[:, :])
            idxB_sb = cpool.tile([128, n_sub * B_CHUNKS * CHUNK // 16], i16)
            nc.sync.dma_start(idxB_sb[:, :], idxB_h[:, :])
            disg = cpool.tile([128, n_grp], f32)
            nc.sync.dma_start(disg[:, :], disg_h[:, :])
            dis2g = cpool.tile([128, n_grp], f32)
            nc.sync.dma_start(dis2g[:, :], dis2g_h[:, :])
            ag = cpool.tile([128, n_grp], f32)
            nc.sync.dma_start(ag[:, :], ag_h[:, :])
            W_sb = cpool.tile([IN_C, HID], f32)
            nc.sync.dma_start(W_sb[:, :], W_h[:, :])
            bX_sb = cpool.tile([128, HID], f32)
            nc.sync.dma_start(bX_sb[:, :], bX_h[:, :])
            Wp_sb = cpool.tile([HID, OUT_C], f32)
            nc.sync.dma_start(Wp_sb[:, :], Wp_h[:, :])
            bP_sb = cpool.tile([128, OUT_C], f32)
            nc.sync.dma_start(bP_sb[:, :], bP_h[:, :])
            ident = cpool.tile([128, 128], f32)
            nc.sync.dma_start(ident[:, :], id_h[:, :])
            xX_sb = cpool.tile([128, n_grp * HID], f32)     # computed below
            dxX_sb = cpool.tile([128, n_grp * HID], f32)    # disg * xX

            # ---- prologue: xX = x @ W + b; T0 = dis * xX -> shard -> AG
            for g in range(n_grp):
                rows = slice(g * 128, (g + 1) * 128)
                gc = slice(g * HID, (g + 1) * HID)
                x_t = wpool.tile([128, IN_C], f32, tag="xt")
                nc.sync.dma_start(x_t[:, :], x_slot_h[rows, :])
                tp_ps = tpool.tile([128, 128], f32, tag="tp")
                nc.tensor.transpose(tp_ps[:, :], x_t[:, :], ident[:, :])
                xT_sb = wpool.tile([128, 128], f32, tag="xT")
                nc.vector.tensor_copy(xT_sb[:, :], tp_ps[:, :])
                mm_ps = tpool.tile([128, HID], f32, tag="mm2")
                nc.tensor.matmul(mm_ps[:, :], xT_sb[:, :], W_sb[:, :],
                                 start=True, stop=True)
                nc.vector.tensor_tensor(xX_sb[:, gc], mm_ps[:, :],
                                        bX_sb[:, :], op=ADD)
                nc.vector.tensor_scalar_mul(dxX_sb[:, gc], xX_sb[:, gc],
                                            disg[:, g:g + 1])
                stage = spool.tile([128, HID], f16, tag="stage")
                nc.vector.tensor_copy(stage[:, :], dxX_sb[:, gc])
                stage_dma(stage, g, 0)

            # ---- 8 SpMM iterations
            # Each iteration emits its own table AllGathers up front, with a
            # runway of PRE_A pre-emitted A-gathers between AG-L and AG-H so
            # the in-order gpsimd queue keeps generating descriptors while
            # AG-H waits for the previous iteration's tail stage writes.
            PRE_A = 6

            def emit_A(b, p):
                g0, ng = batches[b]
                qA = (2 * b) % NQUEUES
                ch0 = g0 * GROUP_SUBS * A_CHUNKS
                nch = ng * GROUP_SUBS * A_CHUNKS
                nidx = nch * CHUNK
                mA = gpool.tile([128, batch_idx], f16, tag="mA")
                nc.gpsimd.dma_gather(
                    mA[:, :nidx].rearrange("p (c e) -> p c e", e=HID),
                    tabL[p][0:half_rows, :],
                    idxA_sb[:, ch0 * (CHUNK // 16):
                            (ch0 + nch) * (CHUNK // 16)],
                    num_idxs=nidx, num_idxs_reg=nidx,
                    elem_size=HID, single_packet=False, queue_num=qA)
                return mA

            for t in range(iters):
                p = t % 2
                last = t == iters - 1
                allgather(shardL[p], tabL[p])
                preA = {b: emit_A(b, p) for b in range(PRE_A)}
                allgather(shardH[p], tabH[p])
                for b, (g0, ng) in enumerate(batches):
                    qB = (2 * b + 1) % NQUEUES
                    ch0 = g0 * GROUP_SUBS * A_CHUNKS        # first A chunk
                    nch = ng * GROUP_SUBS * A_CHUNKS        # chunks (=A=B)
                    nidx = nch * CHUNK
                    mA = preA.pop(b) if b in preA else emit_A(b, p)
                    mB = gpoolB.tile([128, batch_idx], f16, tag="mB")
                    nc.gpsimd.dma_gather(
                        mB[:, :nidx].rearrange("p (c e) -> p c e", e=HID),
                        tabH[p][0:half_rows, :],
                        idxB_sb[:, ch0 * (CHUNK // 16):
                                (ch0 + nch) * (CHUNK // 16)],
                        num_idxs=nidx, num_idxs_reg=nidx,
                        elem_size=HID, single_packet=False, queue_num=qB)
                    for u in range(ng):
                        g = g0 + u
                        gc = slice(g * HID, (g + 1) * HID)
                        ps = ppool.tile([128, HID], f32, tag="ps")
                        for j in range(GROUP_SUBS):
                            sb = g * GROUP_SUBS + j
                            prange = slice(32 * j, 32 * j + 32)
                            tpos = (0, 32 * j)
                            for k in range(A_CHUNKS):
                                cA = sb * A_CHUNKS + k
                                q = (u * GROUP_SUBS + j) * A_CHUNKS + k
                                nc.tensor.matmul(
                                    ps[prange, :],
                                    S_sb[:, cA * 32:(cA + 1) * 32],
                                    mA[:, q * HID:(q + 1) * HID],
                                    start=(k == 0), stop=False,
                                    tile_position=tpos)
                            for k in range(B_CHUNKS):
                                cB = nchA + sb * B_CHUNKS + k
                                q = (u * GROUP_SUBS + j) * B_CHUNKS + k
                                nc.tensor.matmul(
                                    ps[prange, :],
                                    S_sb[:, cB * 32:(cB + 1) * 32],
                                    mB[:, q * HID:(q + 1) * HID],
                                    start=False, stop=(k == B_CHUNKS - 1),
                                    tile_position=tpos)
                        if not last:
                            stage = spool.tile([128, HID], f16, tag="stage")
                            nc.vector.scalar_tensor_tensor(
                                stage[:, :], ps[:, :], dis2g[:, g:g + 1],
                                dxX_sb[:, gc], op0=MULT, op1=ADD)
                            stage_dma(stage, g, (t + 1) % 2)
                        else:
                            u_t = wpool.tile([128, HID], f32, tag="t1")
                            nc.vector.scalar_tensor_tensor(
                                u_t[:, :], ps[:, :], ag[:, g:g + 1],
                                xX_sb[:, gc], op0=MULT, op1=ADD)
                            nc.vector.tensor_scalar_max(u_t[:, :], u_t[:, :],
                                                        0.0)
                            tp_ps = tpool.tile([128, 128], f32, tag="tp")
                            nc.tensor.transpose(tp_ps[:, :], u_t[:, :],
                                                ident[:, :])
                            uT_sb = wpool.tile([128, 128], f32, tag="xT")
                            nc.vector.tensor_copy(uT_sb[:, :], tp_ps[:, :])
                            o_ps = tpool.tile([128, OUT_C], f32, tag="mm2")
                            nc.tensor.matmul(o_ps[:, :], uT_sb[:, :],
                                             Wp_sb[:, :], start=True, stop=True)
                            o_sb = spool.tile([128, OUT_C], f32, tag="osb")
                            nc.vector.tensor_tensor(o_sb[:, :], o_ps[:, :],
                                                    bP_sb[:, :], op=ADD)
                            rows = slice(g * 128, (g + 1) * 128)
                            nc.sync.dma_start(out_h[rows, :], o_sb[:, :])


    nc.compile()
    return nc


# ----------------------------------------------------------------------------
# Runner
# ----------------------------------------------------------------------------

def _run(inputs, n=N, ncores=NCORES, trace=False, use_sim=False, iters=POWER1):
    meta, per_core = _preprocess(inputs, n=n, ncores=ncores)
    nc = _build_program(meta, iters=iters)
    in_maps = [dict(pc) for pc in per_core]

    if use_sim:
        from concourse.bass_interp import MultiCoreSim
        sim = MultiCoreSim(nc, num_cores=ncores)
        for c in range(ncores):
            for k, v in in_maps[c].items():
                sim.cores[c].tensor(k)[:] = v
        sim.simulate(check_with_hw=False)
        results = [{"out": np.array(sim.cores[c].tensor("out"))}
                   for c in range(ncores)]
        bres = None
    else:
        from concourse.bass_utils import run_bass_kernel_spmd
        bres = run_bass_kernel_spmd(nc, in_maps, core_ids=list(range(ncores)),
                                    trace=trace)
        results = bres.results

    # unshard: slots -> nodes
    npc, slots = meta["npc"], meta["slots"]
    son = meta["slot_of_node"]
    out = np.zeros((n, OUT_C), dtype=np.float32)
    for c in range(ncores):
        nodes = np.arange(c * npc, (c + 1) * npc)
        out[nodes] = results[c]["out"][son[nodes]]
    return out, bres


def kernel(**inputs) -> np.ndarray:
    # Run twice and compare: guards against rare transient device faults
    # (observed once after an unrecoverable-NRT event on a shared terminal).
    out1, _ = _run(inputs)
    out2, _ = _run(inputs)
    if np.allclose(out1, out2, rtol=0, atol=1e-4):
        return out1
    out3, _ = _run(inputs)
    if np.allclose(out1, out3, rtol=0, atol=1e-4):
        return out1
    return out2 if np.allclose(out2, out3, rtol=0, atol=1e-4) else out3


# revision 18
# speedup vs baseline: 1.0092x; 1.0092x over previous
"""GCN-style 8-step SpMM power iteration on 8 Trainium2 NeuronCores.

Math (reference):
    deg = segment_sum(1, col); dis = rsqrt(max(deg,1)) where deg>0 else 0
    norm_e = dis[row_e] * dis[col_e];  row' = row - row.min()
    xX = x @ W_linX + b_linX
    hX_{t+1}[v] = sum_{e: row'_e = v} norm_e * hX_t[col_e] + xX[v]   (8 times)
    out = relu(pp0*xX + pp1*hX_8) @ W_pred + b_pred

Key algebraic trick: norm factorizes per-edge into src/dst node factors, so we
keep the node table pre-scaled: T = dis ⊙ hX. Then one step is
    S[v]   = sum_{e->v} T[col_e]            (pure gather + segment-sum, no
                                             per-edge arithmetic at all)
    T'[v]  = (dis*dis_sh)[v]*S[v] + (dis*xX)[v]   (one DVE op per group)

Distribution: nodes dst-sharded over 8 cores. Each core owns a contiguous
slice of a relabeled "slot" table; per-iteration AllGather rebuilds the full
table on every core. The AllGather is split in two halves (slots L/H per
core) so the first half can be exchanged while the second half still
computes. Gather of source rows uses dma_gather (int16 indices -> table
split in L/H halves; edges partitioned by source slot-half). The gathers are
spread across all 4 SWDGE queues (each queue is served by its own pair of
GPSIMD Q7 cores -> 4x descriptor-generation throughput; descriptor
generation is the kernel's bottleneck). Segment-sum runs on the
TensorEngine: edges are binned into sub-blocks of <=32 destination nodes
with a fixed budget of 2 chunks (128 edges each) per source-half; each
chunk's 0/1 selection matrix S (fp16, host-built) is the stationary matmul
operand, the gathered fp16 messages the moving one, accumulating fp32 in
PSUM.
"""

import numpy as np

# problem shape (hardcoded per the task contract)
N = 50000
E = 800000
IN_C = 128
HID = 128
OUT_C = 40
POWER1 = 8

NCORES = 8
SUB_NODES = 32          # destination slots per sub-block (= matmul M)
CHUNK = 128             # edges per chunk (= matmul K)
A_CHUNKS = 2            # chunks per sub-block from source half L
B_CHUNKS = 2
BATCH_GROUPS = 3        # psum groups (of 4 sub-blocks) per gather batch
GROUP_SUBS = 4          # sub-blocks per psum group ([128,128] psum tile)
NQUEUES = 4             # SWDGE queues used for the gathers


# ----------------------------------------------------------------------------
# Host-side preprocessing
# ----------------------------------------------------------------------------

def _pack_core(degA, degB, capA, capB, sub_nodes):
    """2D best-fit-decreasing bin packing of nodes into sub-blocks."""
    order = np.argsort(-np.maximum(degA, degB), kind="stable")
    bins = []        # (node_list, sumA, sumB)
    for v in order:
        a, b = int(degA[v]), int(degB[v])
        best, best_slack = -1, None
        for i, (nodes, sa, sb) in enumerate(bins):
            if len(nodes) < sub_nodes and sa + a <= capA and sb + b <= capB:
                slack = (capA - sa - a) + (capB - sb - b)
                if best_slack is None or slack < best_slack:
                    best, best_slack = i, slack
        if best < 0:
            bins.append(([v], a, b))
        else:
            nodes, sa, sb = bins[best]
            nodes.append(v)
            bins[best] = (nodes, sa + a, sb + b)
    return [b[0] for b in bins]


def _preprocess(inputs, n=N, ncores=NCORES):
    x = np.asarray(inputs["x"], dtype=np.float32)
    edge_index = np.asarray(inputs["edge_index"])
    W_linX = np.asarray(inputs["W_linX"], dtype=np.float32)
    b_linX = np.asarray(inputs["b_linX"], dtype=np.float32)
    policy = np.asarray(inputs["policy"], dtype=np.float64)
    W_pred = np.asarray(inputs["W_pred"], dtype=np.float32)
    b_pred = np.asarray(inputs["b_pred"], dtype=np.float32)

    npc = n // ncores
    half_npc = npc // 2
    row = edge_index[0].astype(np.int64)
    col = edge_index[1].astype(np.int64)
    deg = np.bincount(col, minlength=n).astype(np.float64)
    dis = np.where(deg > 0, 1.0 / np.sqrt(np.maximum(deg, 1.0)), 0.0)
    shift = int(row.min())
    dst = row - shift                      # aggregation destination
    # per-dst factor is dis at the *unshifted* row id
    dis_sh = np.zeros(n, dtype=np.float64)
    hi = n - shift
    dis_sh[:hi] = dis[shift:]

    e = np.exp(policy[:2] - policy[:2].max())
    pp = e / e.sum()
    pp0, pp1 = float(pp[0]), float(pp[1])
    b_comb = pp0 + pp1                      # == 1.0, but don't rely on it

    # a-priori source-half assignment: node v is in half L iff its local id
    # within its core is < npc/2. Needed before packing so per-dst source
    # half degrees are known.
    loc = np.arange(n) % npc
    node_isL = loc < half_npc

    capA, capB = A_CHUNKS * CHUNK, B_CHUNKS * CHUNK

    cores = []
    max_bins_L = max_bins_H = 0
    for c in range(ncores):
        m = (dst >= c * npc) & (dst < (c + 1) * npc)
        e_dst = dst[m] - c * npc
        e_src = col[m]
        srcL = node_isL[e_src]
        degA = np.bincount(e_dst[srcL], minlength=npc)
        degB = np.bincount(e_dst[~srcL], minlength=npc)
        # pack the core's own L-nodes into the first-half bins and H-nodes
        # into the second-half bins (a node's slot half must match its
        # a-priori source-half so consumers know which table to gather from)
        binsL = _pack_core(degA[:half_npc], degB[:half_npc], capA, capB,
                           SUB_NODES)
        binsH = _pack_core(degA[half_npc:], degB[half_npc:], capA, capB,
                           SUB_NODES)
        binsH = [[v + half_npc for v in b] for b in binsH]
        max_bins_L = max(max_bins_L, len(binsL))
        max_bins_H = max(max_bins_H, len(binsH))
        cores.append((e_dst, e_src, binsL, binsH))

    # per-half sub-block count, aligned so psum groups (GROUP_SUBS sub-blocks)
    # do not straddle the half boundary; batches may be ragged.
    n_sub_h = -(-max(max_bins_L, max_bins_H) // GROUP_SUBS) * GROUP_SUBS
    n_sub = 2 * n_sub_h
    slots = n_sub * SUB_NODES               # table rows per core
    hslots = slots // 2                     # rows per half per core
    half_rows = hslots * ncores             # rows of each shared table half
    assert half_rows <= 32767, f"table half {half_rows} exceeds int16 range"
    n_grp = slots // 128

    # slot assignment + per-half global row map
    slot_of_node = np.full(n, -1, dtype=np.int64)   # local slot within core
    for c, (e_dst, e_src, binsL, binsH) in enumerate(cores):
        for bi, nodes in enumerate(binsL):
            for k, v in enumerate(nodes):
                slot_of_node[c * npc + v] = bi * SUB_NODES + k
        for bi, nodes in enumerate(binsH):
            for k, v in enumerate(nodes):
                slot_of_node[c * npc + v] = hslots + bi * SUB_NODES + k
    # global row in the (L|H) shared table
    core_of = np.arange(n) // npc
    lslot = slot_of_node
    is_slotL = (lslot >= 0) & (lslot < hslots)
    grow = np.where(is_slotL, core_of * hslots + lslot,
                    core_of * hslots + (lslot - hslots))

    # an empty slot per half (of core 0) for padding gathers (zero rows)
    padL = padH = -1
    usedL = np.zeros(hslots, dtype=bool)
    usedH = np.zeros(hslots, dtype=bool)
    sl0 = slot_of_node[0:npc]
    usedL[sl0[(sl0 >= 0) & (sl0 < hslots)]] = True
    usedH[sl0[sl0 >= hslots] - hslots] = True
    freeL = np.flatnonzero(~usedL)
    freeH = np.flatnonzero(~usedH)
    assert freeL.size > 0 and freeH.size > 0
    padL, padH = int(freeL[0]), int(freeH[0])

    per_core = []
    nchA = n_sub * A_CHUNKS
    nchB = n_sub * B_CHUNKS
    for c, (e_dst, e_src, binsL, binsH) in enumerate(cores):
        dloc = slot_of_node[e_dst + c * npc]
        dst_bin = np.where(dloc < hslots, dloc // SUB_NODES,
                           n_sub_h + (dloc - hslots) // SUB_NODES)
        dst_k = dloc % SUB_NODES
        isA = node_isL[e_src]
        S = np.zeros((nchA + nchB, CHUNK, SUB_NODES), dtype=np.float16)
        idxA = np.full(nchA * CHUNK, padL, dtype=np.int64)
        idxB = np.full(nchB * CHUNK, padH, dtype=np.int64)
        for bi in range(n_sub):
            for half in (True, False):
                sel = (dst_bin == bi) & (isA == half)
                srcs = e_src[sel]
                dks = dst_k[sel]
                kk = len(srcs)
                cap = capA if half else capB
                assert kk <= cap, (c, bi, half, kk)
                if half:
                    cbase, sbase, idx = bi * A_CHUNKS, 0, idxA
                else:
                    cbase, sbase, idx = bi * B_CHUNKS, nchA, idxB
                for j in range(kk):
                    ch = cbase + j // CHUNK
                    S[sbase + ch, j % CHUNK, dks[j]] = 1.0
                idx[cbase * CHUNK: cbase * CHUNK + kk] = grow[srcs]
        assert idxA.min() >= 0 and idxA.max() < half_rows
        assert idxB.min() >= 0 and idxB.max() < half_rows

        def wrap_idx(idx):
            # index i consumed from [i % 16, i // 16]; replicate to 128 parts
            w = idx.reshape(-1, 16).T.astype(np.int16)      # [16, n/16]
            return np.tile(w, (8, 1))                        # [128, n/16]

        # slot-layout host arrays
        x_slot = np.zeros((slots, IN_C), dtype=np.float32)
        dis_slot = np.zeros(slots, dtype=np.float64)
        dsh_slot = np.zeros(slots, dtype=np.float64)
        nodes_c = np.arange(c * npc, (c + 1) * npc)
        sl = slot_of_node[nodes_c]
        x_slot[sl] = x[nodes_c]
        dis_slot[sl] = dis[nodes_c]
        dsh_slot[sl] = dis_sh[nodes_c]

        grp = lambda v: v.reshape(n_grp, 128).T.astype(np.float32)
        per_core.append({
            "x_slot": x_slot,
            "S": np.ascontiguousarray(
                S.transpose(1, 0, 2).reshape(CHUNK, -1)),    # [128, TC*32]
            "idxA": wrap_idx(idxA),
            "idxB": wrap_idx(idxB),
            "disg": grp(dis_slot),
            "dis2g": grp(dis_slot * dsh_slot),
            "ag": grp((pp1 / b_comb) * dsh_slot),
            "W_linX": W_linX,
            "bX": np.tile(b_linX[None, :], (128, 1)).astype(np.float32),
            "W_pred": (b_comb * W_pred).astype(np.float32),
            "bP": np.tile(b_pred[None, :], (128, 1)).astype(np.float32),
            "ident": np.eye(128, dtype=np.float32),
        })

    meta = dict(n=n, ncores=ncores, npc=npc, n_sub=n_sub, n_sub_h=n_sub_h,
                n_grp=n_grp, slots=slots, hslots=hslots, half_rows=half_rows,
                slot_of_node=slot_of_node)
    return meta, per_core


# ----------------------------------------------------------------------------
# Bass program
# ----------------------------------------------------------------------------

def _build_program(meta, iters=POWER1):
    import concourse.bacc as bacc
    import concourse.mybir as mybir
    from concourse import tile

    f32, f16, i16 = mybir.dt.float32, mybir.dt.float16, mybir.dt.int16
    ADD, MULT = mybir.AluOpType.add, mybir.AluOpType.mult

    ncores = meta["ncores"]
    n_sub, n_grp = meta["n_sub"], meta["n_grp"]
    slots, hslots = meta["slots"], meta["hslots"]
    half_rows = meta["half_rows"]
    nchA = n_sub * A_CHUNKS
    batch_idx = BATCH_GROUPS * GROUP_SUBS * A_CHUNKS * CHUNK
    TC = n_sub * (A_CHUNKS + B_CHUNKS)
    hgrp = n_grp // 2
    # ragged batches: BATCH_GROUPS psum groups each, not straddling halves
    batches = []
    for h in range(2):
        g = h * hgrp
        while g < (h + 1) * hgrp:
            k = min(BATCH_GROUPS, (h + 1) * hgrp - g)
            batches.append((g, k))
            g += k

    nc = bacc.Bacc("TRN2", target_bir_lowering=False, debug=False,
                   enable_asserts=False, num_devices=ncores,
                   num_swdge_queues=NQUEUES,
                   dynamic_dma_scratch_size=32768)

    x_slot_h = nc.dram_tensor("x_slot", [slots, IN_C], f32, kind="ExternalInput")
    S_h = nc.dram_tensor("S", [CHUNK, TC * SUB_NODES], f16, kind="ExternalInput")
    idxA_h = nc.dram_tensor("idxA", [128, nchA * CHUNK // 16], i16,
                            kind="ExternalInput")
    idxB_h = nc.dram_tensor("idxB", [128, n_sub * B_CHUNKS * CHUNK // 16], i16,
                            kind="ExternalInput")
    disg_h = nc.dram_tensor("disg", [128, n_grp], f32, kind="ExternalInput")
    dis2g_h = nc.dram_tensor("dis2g", [128, n_grp], f32, kind="ExternalInput")
    ag_h = nc.dram_tensor("ag", [128, n_grp], f32, kind="ExternalInput")
    W_h = nc.dram_tensor("W_linX", [IN_C, HID], f32, kind="ExternalInput")
    bX_h = nc.dram_tensor("bX", [128, HID], f32, kind="ExternalInput")
    Wp_h = nc.dram_tensor("W_pred", [HID, OUT_C], f32, kind="ExternalInput")
    bP_h = nc.dram_tensor("bP", [128, OUT_C], f32, kind="ExternalInput")
    id_h = nc.dram_tensor("ident", [128, 128], f32, kind="ExternalInput")

    # ping-pong shared tables, split in L/H halves so the L AllGather can
    # launch while H still computes
    tabL = [nc.dram_tensor(f"tabL{p}", [half_rows, HID], f16,
                           addr_space="Shared") for p in range(2)]
    tabH = [nc.dram_tensor(f"tabH{p}", [half_rows, HID], f16,
                           addr_space="Shared") for p in range(2)]
    shardL = [nc.dram_tensor(f"shardL{p}", [hslots, HID], f16)
              for p in range(2)]
    shardH = [nc.dram_tensor(f"shardH{p}", [hslots, HID], f16)
              for p in range(2)]
    out_h = nc.dram_tensor("out", [slots, OUT_C], f32, kind="ExternalOutput")

    rg = [list(range(ncores))]

    def stage_dma(stage, g, parity):
        # route group g's staged rows to the right shard half
        if g < hgrp:
            rows = slice(g * 128, (g + 1) * 128)
            nc.sync.dma_start(shardL[parity][rows, :], stage[:, :])
        else:
            rows = slice((g - hgrp) * 128, (g - hgrp + 1) * 128)
            nc.sync.dma_start(shardH[parity][rows, :], stage[:, :])

    def allgather(src, dst):
        nc.gpsimd.collective_compute(
            "AllGather", mybir.AluOpType.bypass, replica_groups=rg,
            ins=[src.ap().opt()], outs=[dst.ap().opt()])

    with tile.TileContext(nc, num_cores=ncores) as tc:
        import contextlib
        with contextlib.ExitStack() as ctx:
            cpool = ctx.enter_context(tc.tile_pool(name="const", bufs=1))
            wpool = ctx.enter_context(tc.tile_pool(name="work", bufs=2))
            gpool = ctx.enter_context(tc.tile_pool(name="gathA", bufs=6))
            gpoolB = ctx.enter_context(tc.tile_pool(name="gathB", bufs=3))
            spool = ctx.enter_context(tc.tile_pool(name="stage", bufs=3))
            ppool = ctx.enter_context(
                tc.tile_pool(name="psum", bufs=6, space="PSUM"))
            tpool = ctx.enter_context(
                tc.tile_pool(name="psum2", bufs=1, space="PSUM"))

            # persistent SBUF
            S_sb = cpool.tile([CHUNK, TC * SUB_NODES], f16)
            nc.sync.dma_start(S_sb[:, :], S_h[:, :])
            idxA_sb = cpool.tile([128, nchA * CHUNK // 16], i16)
            nc.sync.dma_start(idxA_sb[:, :], idxA_h[:, :])
            idxB_sb = cpool.tile([128, n_sub * B_CHUNKS * CHUNK // 16], i16)
            nc.sync.dma_start(idxB_sb[:, :], idxB_h[:, :])
            disg = cpool.tile([128, n_grp], f32)
            nc.sync.dma_start(disg[:, :], disg_h[:, :])
            dis2g = cpool.tile([128, n_grp], f32)
            nc.sync.dma_start(dis2g[:, :], dis2g_h[:, :])
            ag = cpool.tile([128, n_grp], f32)
            nc.sync.dma_start(ag[:, :], ag_h[:, :])
            W_sb = cpool.tile([IN_C, HID], f32)
            nc.sync.dma_start(W_sb[:, :], W_h[:, :])
            bX_sb = cpool.tile([128, HID], f32)
            nc.sync.dma_start(bX_sb[:, :], bX_h[:, :])
            Wp_sb = cpool.tile([HID, OUT_C], f32)
            nc.sync.dma_start(Wp_sb[:, :], Wp_h[:, :])
            bP_sb = cpool.tile([128, OUT_C], f32)
            nc.sync.dma_start(bP_sb[:, :], bP_h[:, :])
            ident = cpool.tile([128, 128], f32)
            nc.sync.dma_start(ident[:, :], id_h[:, :])
            xX_sb = cpool.tile([128, n_grp * HID], f32)     # computed below
            dxX_sb = cpool.tile([128, n_grp * HID], f32)    # disg * xX

            # ---- prologue: xX = x @ W + b; T0 = dis * xX -> shard -> AG
            for g in range(n_grp):
                rows = slice(g * 128, (g + 1) * 128)
                gc = slice(g * HID, (g + 1) * HID)
                x_t = wpool.tile([128, IN_C], f32, tag="xt")
                nc.sync.dma_start(x_t[:, :], x_slot_h[rows, :])
                tp_ps = tpool.tile([128, 128], f32, tag="tp")
                nc.tensor.transpose(tp_ps[:, :], x_t[:, :], ident[:, :])
                xT_sb = wpool.tile([128, 128], f32, tag="xT")
                nc.vector.tensor_copy(xT_sb[:, :], tp_ps[:, :])
                mm_ps = tpool.tile([128, HID], f32, tag="mm2")
                nc.tensor.matmul(mm_ps[:, :], xT_sb[:, :], W_sb[:, :],
                                 start=True, stop=True)
                nc.vector.tensor_tensor(xX_sb[:, gc], mm_ps[:, :],
                                        bX_sb[:, :], op=ADD)
                nc.vector.tensor_scalar_mul(dxX_sb[:, gc], xX_sb[:, gc],
                                            disg[:, g:g + 1])
                stage = spool.tile([128, HID], f16, tag="stage")
                nc.vector.tensor_copy(stage[:, :], dxX_sb[:, gc])
                stage_dma(stage, g, 0)
                if g == hgrp - 1:
                    allgather(shardL[0], tabL[0])

            # ---- 8 SpMM iterations
            # Each iteration emits its own table AllGathers up front, with a
            # runway of PRE_A pre-emitted A-gathers between AG-L and AG-H so
            # the in-order gpsimd queue keeps generating descriptors while
            # AG-H waits for the previous iteration's tail stage writes.
            PRE_A = 6

            def emit_A(b, p):
                g0, ng = batches[b]
                qA = (2 * b) % NQUEUES
                ch0 = g0 * GROUP_SUBS * A_CHUNKS
                nch = ng * GROUP_SUBS * A_CHUNKS
                nidx = nch * CHUNK
                mA = gpool.tile([128, batch_idx], f16, tag="mA")
                nc.gpsimd.dma_gather(
                    mA[:, :nidx].rearrange("p (c e) -> p c e", e=HID),
                    tabL[p][0:half_rows, :],
                    idxA_sb[:, ch0 * (CHUNK // 16):
                            (ch0 + nch) * (CHUNK // 16)],
                    num_idxs=nidx, num_idxs_reg=nidx,
                    elem_size=HID, single_packet=False, queue_num=qA)
                return mA

            n_hbat = sum(1 for g0, ng in batches if g0 < hgrp)
            for t in range(iters):
                p = t % 2
                last = t == iters - 1
                preA = {b: emit_A(b, p) for b in range(PRE_A)}
                allgather(shardH[p], tabH[p])
                for b, (g0, ng) in enumerate(batches):
                    qB = (2 * b + 1) % NQUEUES
                    ch0 = g0 * GROUP_SUBS * A_CHUNKS        # first A chunk
                    nch = ng * GROUP_SUBS * A_CHUNKS        # chunks (=A=B)
                    nidx = nch * CHUNK
                    mA = preA.pop(b) if b in preA else emit_A(b, p)
                    mB = gpoolB.tile([128, batch_idx], f16, tag="mB")
                    nc.gpsimd.dma_gather(
                        mB[:, :nidx].rearrange("p (c e) -> p c e", e=HID),
                        tabH[p][0:half_rows, :],
                        idxB_sb[:, ch0 * (CHUNK // 16):
                                (ch0 + nch) * (CHUNK // 16)],
                        num_idxs=nidx, num_idxs_reg=nidx,
                        elem_size=HID, single_packet=False, queue_num=qB)
                    for u in range(ng):
                        g = g0 + u
                        gc = slice(g * HID, (g + 1) * HID)
                        ps = ppool.tile([128, HID], f32, tag="ps")
                        for j in range(GROUP_SUBS):
                            sb = g * GROUP_SUBS + j
                            prange = slice(32 * j, 32 * j + 32)
                            tpos = (0, 32 * j)
                            for k in range(A_CHUNKS):
                                cA = sb * A_CHUNKS + k
                                q = (u * GROUP_SUBS + j) * A_CHUNKS + k
                                nc.tensor.matmul(
                                    ps[prange, :],
                                    S_sb[:, cA * 32:(cA + 1) * 32],
                                    mA[:, q * HID:(q + 1) * HID],
                                    start=(k == 0), stop=False,
                                    tile_position=tpos)
                            for k in range(B_CHUNKS):
                                cB = nchA + sb * B_CHUNKS + k
                                q = (u * GROUP_SUBS + j) * B_CHUNKS + k
                                nc.tensor.matmul(
                                    ps[prange, :],
                                    S_sb[:, cB * 32:(cB + 1) * 32],
                                    mB[:, q * HID:(q + 1) * HID],
                                    start=False, stop=(k == B_CHUNKS - 1),
                                    tile_position=tpos)
                        if not last:
                            stage = spool.tile([128, HID], f16, tag="stage")
                            nc.vector.scalar_tensor_tensor(
                                stage[:, :], ps[:, :], dis2g[:, g:g + 1],
                                dxX_sb[:, gc], op0=MULT, op1=ADD)
                            stage_dma(stage, g, (t + 1) % 2)
                        else:
                            u_t = wpool.tile([128, HID], f32, tag="t1")
                            nc.vector.scalar_tensor_tensor(
                                u_t[:, :], ps[:, :], ag[:, g:g + 1],
                                xX_sb[:, gc], op0=MULT, op1=ADD)
                            nc.vector.tensor_scalar_max(u_t[:, :], u_t[:, :],
                                                        0.0)
                            tp_ps = tpool.tile([128, 128], f32, tag="tp")
                            nc.tensor.transpose(tp_ps[:, :], u_t[:, :],
                                                ident[:, :])
                            uT_sb = wpool.tile([128, 128], f32, tag="xT")
                            nc.vector.tensor_copy(uT_sb[:, :], tp_ps[:, :])
                            o_ps = tpool.tile([128, OUT_C], f32, tag="mm2")
                            nc.tensor.matmul(o_ps[:, :], uT_sb[:, :],
                                             Wp_sb[:, :], start=True, stop=True)
                            o_sb = spool.tile([128, OUT_C], f32, tag="osb")
                            nc.vector.tensor_tensor(o_sb[:, :], o_ps[:, :],
                                                    bP_sb[:, :], op=ADD)
                            rows = slice(g * 128, (g + 1) * 128)
                            nc.sync.dma_start(out_h[rows, :], o_sb[:, :])
                    if not last and b == n_hbat + 2:
                        allgather(shardL[(t + 1) % 2], tabL[(t + 1) % 2])


    nc.compile()
    return nc


# ----------------------------------------------------------------------------
# Runner
# ----------------------------------------------------------------------------

def _run(inputs, n=N, ncores=NCORES, trace=False, use_sim=False, iters=POWER1):
    meta, per_core = _preprocess(inputs, n=n, ncores=ncores)
    nc = _build_program(meta, iters=iters)
    in_maps = [dict(pc) for pc in per_core]

    if use_sim:
        from concourse.bass_interp import MultiCoreSim
        sim = MultiCoreSim(nc, num_cores=ncores)
        for c in range(ncores):
            for k, v in in_maps[c].items():
                sim.cores[c].tensor(k)[:] = v
        sim.simulate(check_with_hw=False)
        results = [{"out": np.array(sim.cores[c].tensor("out"))}
                   for c in range(ncores)]
        bres = None
    else:
        from concourse.bass_utils import run_bass_kernel_spmd
        bres = run_bass_kernel_spmd(nc, in_maps, core_ids=list(range(ncores)),
                                    trace=trace)
        results = bres.results

    # unshard: slots -> nodes
    npc, slots = meta["npc"], meta["slots"]
    son = meta["slot_of_node"]
    out = np.zeros((n, OUT_C), dtype=np.float32)
    for c in range(ncores):
        nodes = np.arange(c * npc, (c + 1) * npc)
        out[nodes] = results[c]["out"][son[nodes]]
    return out, bres


def kernel(**inputs) -> np.ndarray:
    # Run twice and compare: guards against rare transient device faults
    # (observed once after an unrecoverable-NRT event on a shared terminal).
    out1, _ = _run(inputs)
    out2, _ = _run(inputs)
    if np.allclose(out1, out2, rtol=0, atol=1e-4):
        return out1
    out3, _ = _run(inputs)
    if np.allclose(out1, out3, rtol=0, atol=1e-4):
        return out1
    return out2 if np.allclose(out2, out3, rtol=0, atol=1e-4) else out3


# revision 25
# speedup vs baseline: 1.0968x; 1.0868x over previous
"""GCN-style 8-step SpMM power iteration on 8 Trainium2 NeuronCores.

Math (reference):
    deg = segment_sum(1, col); dis = rsqrt(max(deg,1)) where deg>0 else 0
    norm_e = dis[row_e] * dis[col_e];  row' = row - row.min()
    xX = x @ W_linX + b_linX
    hX_{t+1}[v] = sum_{e: row'_e = v} norm_e * hX_t[col_e] + xX[v]   (8 times)
    out = relu(pp0*xX + pp1*hX_8) @ W_pred + b_pred

Key algebraic trick: norm factorizes per-edge into src/dst node factors, so we
keep the node table pre-scaled: T = dis ⊙ hX. Then one step is
    S[v]   = sum_{e->v} T[col_e]            (pure gather + segment-sum, no
                                             per-edge arithmetic at all)
    T'[v]  = (dis*dis_sh)[v]*S[v] + (dis*xX)[v]   (one DVE op per group)

Distribution: nodes dst-sharded over 8 cores. Each core owns a contiguous
slice of a relabeled "slot" table; per-iteration AllGather rebuilds the full
table on every core. The AllGather is split in two halves (slots L/H per
core) so the first half can be exchanged while the second half still
computes. Gather of source rows uses dma_gather (int16 indices -> table
split in L/H halves; edges partitioned by source slot-half). The gathers are
spread across all 4 SWDGE queues (each queue is served by its own pair of
GPSIMD Q7 cores -> 4x descriptor-generation throughput; descriptor
generation is the kernel's bottleneck). Segment-sum runs on the
TensorEngine: edges are binned into sub-blocks of <=32 destination nodes
with a fixed budget of 2 chunks (128 edges each) per source-half; each
chunk's 0/1 selection matrix S (fp16, host-built) is the stationary matmul
operand, the gathered fp16 messages the moving one, accumulating fp32 in
PSUM.
"""

import numpy as np

# problem shape (hardcoded per the task contract)
N = 50000
E = 800000
IN_C = 128
HID = 128
OUT_C = 40
POWER1 = 8

NCORES = 8
SUB_NODES = 32          # destination slots per sub-block (= matmul M)
CHUNK = 128             # edges per chunk (= matmul K)
A_CHUNKS = 2            # chunks per sub-block from source half L
B_CHUNKS = 2
BATCH_GROUPS = 2        # psum groups (of 4 sub-blocks) per gather batch
GROUP_SUBS = 4          # sub-blocks per psum group ([128,128] psum tile)
NQUEUES = 4             # SWDGE queues used for the gathers


# ----------------------------------------------------------------------------
# Host-side preprocessing
# ----------------------------------------------------------------------------

def _pack_core(degA, degB, capA, capB, sub_nodes):
    """2D best-fit-decreasing bin packing of nodes into sub-blocks."""
    order = np.argsort(-np.maximum(degA, degB), kind="stable")
    bins = []        # (node_list, sumA, sumB)
    for v in order:
        a, b = int(degA[v]), int(degB[v])
        best, best_slack = -1, None
        for i, (nodes, sa, sb) in enumerate(bins):
            if len(nodes) < sub_nodes and sa + a <= capA and sb + b <= capB:
                slack = (capA - sa - a) + (capB - sb - b)
                if best_slack is None or slack < best_slack:
                    best, best_slack = i, slack
        if best < 0:
            bins.append(([v], a, b))
        else:
            nodes, sa, sb = bins[best]
            nodes.append(v)
            bins[best] = (nodes, sa + a, sb + b)
    return [b[0] for b in bins]


def _preprocess(inputs, n=N, ncores=NCORES):
    x = np.asarray(inputs["x"], dtype=np.float32)
    edge_index = np.asarray(inputs["edge_index"])
    W_linX = np.asarray(inputs["W_linX"], dtype=np.float32)
    b_linX = np.asarray(inputs["b_linX"], dtype=np.float32)
    policy = np.asarray(inputs["policy"], dtype=np.float64)
    W_pred = np.asarray(inputs["W_pred"], dtype=np.float32)
    b_pred = np.asarray(inputs["b_pred"], dtype=np.float32)

    npc = n // ncores
    half_npc = npc // 2
    row = edge_index[0].astype(np.int64)
    col = edge_index[1].astype(np.int64)
    deg = np.bincount(col, minlength=n).astype(np.float64)
    dis = np.where(deg > 0, 1.0 / np.sqrt(np.maximum(deg, 1.0)), 0.0)
    shift = int(row.min())
    dst = row - shift                      # aggregation destination
    # per-dst factor is dis at the *unshifted* row id
    dis_sh = np.zeros(n, dtype=np.float64)
    hi = n - shift
    dis_sh[:hi] = dis[shift:]

    e = np.exp(policy[:2] - policy[:2].max())
    pp = e / e.sum()
    pp0, pp1 = float(pp[0]), float(pp[1])
    b_comb = pp0 + pp1                      # == 1.0, but don't rely on it

    # a-priori source-half assignment: node v is in half L iff its local id
    # within its core is < npc/2. Needed before packing so per-dst source
    # half degrees are known.
    loc = np.arange(n) % npc
    node_isL = loc < half_npc

    capA, capB = A_CHUNKS * CHUNK, B_CHUNKS * CHUNK

    cores = []
    max_bins_L = max_bins_H = 0
    for c in range(ncores):
        m = (dst >= c * npc) & (dst < (c + 1) * npc)
        e_dst = dst[m] - c * npc
        e_src = col[m]
        srcL = node_isL[e_src]
        degA = np.bincount(e_dst[srcL], minlength=npc)
        degB = np.bincount(e_dst[~srcL], minlength=npc)
        # pack the core's own L-nodes into the first-half bins and H-nodes
        # into the second-half bins (a node's slot half must match its
        # a-priori source-half so consumers know which table to gather from)
        binsL = _pack_core(degA[:half_npc], degB[:half_npc], capA, capB,
                           SUB_NODES)
        binsH = _pack_core(degA[half_npc:], degB[half_npc:], capA, capB,
                           SUB_NODES)
        binsH = [[v + half_npc for v in b] for b in binsH]
        max_bins_L = max(max_bins_L, len(binsL))
        max_bins_H = max(max_bins_H, len(binsH))
        cores.append((e_dst, e_src, binsL, binsH))

    # per-half sub-block count, aligned so psum groups (GROUP_SUBS sub-blocks)
    # do not straddle the half boundary; batches may be ragged.
    n_sub_h = -(-max(max_bins_L, max_bins_H) // GROUP_SUBS) * GROUP_SUBS
    n_sub = 2 * n_sub_h
    slots = n_sub * SUB_NODES               # table rows per core
    hslots = slots // 2                     # rows per half per core
    half_rows = hslots * ncores             # rows of each shared table half
    assert half_rows <= 32767, f"table half {half_rows} exceeds int16 range"
    n_grp = slots // 128

    # slot assignment + per-half global row map
    slot_of_node = np.full(n, -1, dtype=np.int64)   # local slot within core
    for c, (e_dst, e_src, binsL, binsH) in enumerate(cores):
        for bi, nodes in enumerate(binsL):
            for k, v in enumerate(nodes):
                slot_of_node[c * npc + v] = bi * SUB_NODES + k
        for bi, nodes in enumerate(binsH):
            for k, v in enumerate(nodes):
                slot_of_node[c * npc + v] = hslots + bi * SUB_NODES + k
    # global row in the (L|H) shared table
    core_of = np.arange(n) // npc
    lslot = slot_of_node
    is_slotL = (lslot >= 0) & (lslot < hslots)
    grow = np.where(is_slotL, core_of * hslots + lslot,
                    core_of * hslots + (lslot - hslots))

    # an empty slot per half (of core 0) for padding gathers (zero rows)
    padL = padH = -1
    usedL = np.zeros(hslots, dtype=bool)
    usedH = np.zeros(hslots, dtype=bool)
    sl0 = slot_of_node[0:npc]
    usedL[sl0[(sl0 >= 0) & (sl0 < hslots)]] = True
    usedH[sl0[sl0 >= hslots] - hslots] = True
    freeL = np.flatnonzero(~usedL)
    freeH = np.flatnonzero(~usedH)
    assert freeL.size > 0 and freeH.size > 0
    padL, padH = int(freeL[0]), int(freeH[0])

    per_core = []
    nchA = n_sub * A_CHUNKS
    nchB = n_sub * B_CHUNKS
    for c, (e_dst, e_src, binsL, binsH) in enumerate(cores):
        dloc = slot_of_node[e_dst + c * npc]
        dst_bin = np.where(dloc < hslots, dloc // SUB_NODES,
                           n_sub_h + (dloc - hslots) // SUB_NODES)
        dst_k = dloc % SUB_NODES
        isA = node_isL[e_src]
        S = np.zeros((nchA + nchB, CHUNK, SUB_NODES), dtype=np.float16)
        idxA = np.full(nchA * CHUNK, padL, dtype=np.int64)
        idxB = np.full(nchB * CHUNK, padH, dtype=np.int64)
        for bi in range(n_sub):
            for half in (True, False):
                sel = (dst_bin == bi) & (isA == half)
                srcs = e_src[sel]
                dks = dst_k[sel]
                kk = len(srcs)
                cap = capA if half else capB
                assert kk <= cap, (c, bi, half, kk)
                if half:
                    cbase, sbase, idx = bi * A_CHUNKS, 0, idxA
                else:
                    cbase, sbase, idx = bi * B_CHUNKS, nchA, idxB
                for j in range(kk):
                    ch = cbase + j // CHUNK
                    S[sbase + ch, j % CHUNK, dks[j]] = 1.0
                idx[cbase * CHUNK: cbase * CHUNK + kk] = grow[srcs]
        assert idxA.min() >= 0 and idxA.max() < half_rows
        assert idxB.min() >= 0 and idxB.max() < half_rows

        def wrap_idx(idx):
            # index i consumed from [i % 16, i // 16]; replicate to 128 parts
            w = idx.reshape(-1, 16).T.astype(np.int16)      # [16, n/16]
            return np.tile(w, (8, 1))                        # [128, n/16]

        # slot-layout host arrays (x pre-transposed: [IN_C, slots])
        x_slot = np.zeros((slots, IN_C), dtype=np.float32)
        dis_slot = np.zeros(slots, dtype=np.float64)
        dsh_slot = np.zeros(slots, dtype=np.float64)
        nodes_c = np.arange(c * npc, (c + 1) * npc)
        sl = slot_of_node[nodes_c]
        x_slot[sl] = x[nodes_c]
        dis_slot[sl] = dis[nodes_c]
        dsh_slot[sl] = dis_sh[nodes_c]

        grp = lambda v: v.reshape(n_grp, 128).T.astype(np.float32)
        per_core.append({
            "x_slot": np.ascontiguousarray(x_slot.T),
            "S": np.ascontiguousarray(
                S.transpose(1, 0, 2).reshape(CHUNK, -1)),    # [128, TC*32]
            "idxA": wrap_idx(idxA),
            "idxB": wrap_idx(idxB),
            "disg": grp(dis_slot),
            "dis2g": grp(dis_slot * dsh_slot),
            "ag": grp((pp1 / b_comb) * dsh_slot),
            "W_linX": W_linX,
            "bX": np.tile(b_linX[None, :], (128, 1)).astype(np.float32),
            "W_pred": (b_comb * W_pred).astype(np.float32),
            "bP": np.tile(b_pred[None, :], (128, 1)).astype(np.float32),
            "ident": np.eye(128, dtype=np.float32),
        })

    meta = dict(n=n, ncores=ncores, npc=npc, n_sub=n_sub, n_sub_h=n_sub_h,
                n_grp=n_grp, slots=slots, hslots=hslots, half_rows=half_rows,
                slot_of_node=slot_of_node)
    return meta, per_core


# ----------------------------------------------------------------------------
# Bass program
# ----------------------------------------------------------------------------

def _build_program(meta, iters=POWER1):
    import concourse.bacc as bacc
    import concourse.mybir as mybir
    from concourse import tile

    f32, f16, i16 = mybir.dt.float32, mybir.dt.float16, mybir.dt.int16
    ADD, MULT = mybir.AluOpType.add, mybir.AluOpType.mult

    ncores = meta["ncores"]
    n_sub, n_grp = meta["n_sub"], meta["n_grp"]
    slots, hslots = meta["slots"], meta["hslots"]
    half_rows = meta["half_rows"]
    nchA = n_sub * A_CHUNKS
    batch_idx = BATCH_GROUPS * GROUP_SUBS * A_CHUNKS * CHUNK
    TC = n_sub * (A_CHUNKS + B_CHUNKS)
    hgrp = n_grp // 2
    # ragged batches: BATCH_GROUPS psum groups each, not straddling halves
    batches = []
    for h in range(2):
        g = h * hgrp
        while g < (h + 1) * hgrp:
            k = min(BATCH_GROUPS, (h + 1) * hgrp - g)
            batches.append((g, k))
            g += k

    nc = bacc.Bacc("TRN2", target_bir_lowering=False, debug=False,
                   enable_asserts=False, num_devices=ncores,
                   num_swdge_queues=NQUEUES,
                   dynamic_dma_scratch_size=32768)

    x_slot_h = nc.dram_tensor("x_slot", [IN_C, slots], f32, kind="ExternalInput")
    S_h = nc.dram_tensor("S", [CHUNK, TC * SUB_NODES], f16, kind="ExternalInput")
    idxA_h = nc.dram_tensor("idxA", [128, nchA * CHUNK // 16], i16,
                            kind="ExternalInput")
    idxB_h = nc.dram_tensor("idxB", [128, n_sub * B_CHUNKS * CHUNK // 16], i16,
                            kind="ExternalInput")
    disg_h = nc.dram_tensor("disg", [128, n_grp], f32, kind="ExternalInput")
    dis2g_h = nc.dram_tensor("dis2g", [128, n_grp], f32, kind="ExternalInput")
    ag_h = nc.dram_tensor("ag", [128, n_grp], f32, kind="ExternalInput")
    W_h = nc.dram_tensor("W_linX", [IN_C, HID], f32, kind="ExternalInput")
    bX_h = nc.dram_tensor("bX", [128, HID], f32, kind="ExternalInput")
    Wp_h = nc.dram_tensor("W_pred", [HID, OUT_C], f32, kind="ExternalInput")
    bP_h = nc.dram_tensor("bP", [128, OUT_C], f32, kind="ExternalInput")
    id_h = nc.dram_tensor("ident", [128, 128], f32, kind="ExternalInput")

    # ping-pong shared tables, split in L/H halves so the L AllGather can
    # launch while H still computes
    tabL = [nc.dram_tensor(f"tabL{p}", [half_rows, HID], f16,
                           addr_space="Shared") for p in range(2)]
    tabH = [nc.dram_tensor(f"tabH{p}", [half_rows, HID], f16,
                           addr_space="Shared") for p in range(2)]
    shardL = [nc.dram_tensor(f"shardL{p}", [hslots, HID], f16)
              for p in range(2)]
    shardH = [nc.dram_tensor(f"shardH{p}", [hslots, HID], f16)
              for p in range(2)]
    out_h = nc.dram_tensor("out", [slots, OUT_C], f32, kind="ExternalOutput")

    rg = [list(range(ncores))]

    def stage_dma(stage, g, parity):
        # route group g's staged rows to the right shard half
        if g < hgrp:
            rows = slice(g * 128, (g + 1) * 128)
            nc.sync.dma_start(shardL[parity][rows, :], stage[:, :])
        else:
            rows = slice((g - hgrp) * 128, (g - hgrp + 1) * 128)
            nc.sync.dma_start(shardH[parity][rows, :], stage[:, :])

    def allgather(src, dst):
        nc.gpsimd.collective_compute(
            "AllGather", mybir.AluOpType.bypass, replica_groups=rg,
            ins=[src.ap().opt()], outs=[dst.ap().opt()])

    with tile.TileContext(nc, num_cores=ncores) as tc:
        import contextlib
        with contextlib.ExitStack() as ctx:
            cpool = ctx.enter_context(tc.tile_pool(name="const", bufs=1))
            wpool = ctx.enter_context(tc.tile_pool(name="work", bufs=2))
            gpool = ctx.enter_context(tc.tile_pool(name="gathA", bufs=8))
            gpoolB = ctx.enter_context(tc.tile_pool(name="gathB", bufs=3))
            spool = ctx.enter_context(tc.tile_pool(name="stage", bufs=3))
            ppool = ctx.enter_context(
                tc.tile_pool(name="psum", bufs=6, space="PSUM"))
            tpool = ctx.enter_context(
                tc.tile_pool(name="psum2", bufs=1, space="PSUM"))

            # persistent SBUF
            S_sb = cpool.tile([CHUNK, TC * SUB_NODES], f16)
            nc.sync.dma_start(S_sb[:, :], S_h[:, :])
            idxA_sb = cpool.tile([128, nchA * CHUNK // 16], i16)
            nc.sync.dma_start(idxA_sb[:, :], idxA_h[:, :])
            idxB_sb = cpool.tile([128, n_sub * B_CHUNKS * CHUNK // 16], i16)
            nc.sync.dma_start(idxB_sb[:, :], idxB_h[:, :])
            disg = cpool.tile([128, n_grp], f32)
            nc.sync.dma_start(disg[:, :], disg_h[:, :])
            dis2g = cpool.tile([128, n_grp], f32)
            nc.sync.dma_start(dis2g[:, :], dis2g_h[:, :])
            ag = cpool.tile([128, n_grp], f32)
            nc.sync.dma_start(ag[:, :], ag_h[:, :])
            W_sb = cpool.tile([IN_C, HID], f32)
            nc.sync.dma_start(W_sb[:, :], W_h[:, :])
            bX_sb = cpool.tile([128, HID], f32)
            nc.sync.dma_start(bX_sb[:, :], bX_h[:, :])
            Wp_sb = cpool.tile([HID, OUT_C], f32)
            nc.sync.dma_start(Wp_sb[:, :], Wp_h[:, :])
            bP_sb = cpool.tile([128, OUT_C], f32)
            nc.sync.dma_start(bP_sb[:, :], bP_h[:, :])
            ident = cpool.tile([128, 128], f32)
            nc.sync.dma_start(ident[:, :], id_h[:, :])
            xX_sb = cpool.tile([128, n_grp * HID], f32)     # computed below
            dxX_sb = cpool.tile([128, n_grp * HID], f32)    # disg * xX

            # ---- prologue: xX = x @ W + b; T0 = dis * xX -> shard -> AG
            for g in range(n_grp):
                rows = slice(g * 128, (g + 1) * 128)
                gc = slice(g * HID, (g + 1) * HID)
                xT_sb = wpool.tile([128, 128], f32, tag="xT")
                nc.sync.dma_start(xT_sb[:, :], x_slot_h[:, rows])
                mm_ps = tpool.tile([128, HID], f32, tag="mm2")
                nc.tensor.matmul(mm_ps[:, :], xT_sb[:, :], W_sb[:, :],
                                 start=True, stop=True)
                nc.vector.tensor_tensor(xX_sb[:, gc], mm_ps[:, :],
                                        bX_sb[:, :], op=ADD)
                nc.vector.tensor_scalar_mul(dxX_sb[:, gc], xX_sb[:, gc],
                                            disg[:, g:g + 1])
                stage = spool.tile([128, HID], f16, tag="stage")
                nc.vector.tensor_copy(stage[:, :], dxX_sb[:, gc])
                stage_dma(stage, g, 0)
                if g == hgrp - 1:
                    allgather(shardL[0], tabL[0])

            # ---- 8 SpMM iterations
            # Each iteration emits its own table AllGathers up front, with a
            # runway of PRE_A pre-emitted A-gathers between AG-L and AG-H so
            # the in-order gpsimd queue keeps generating descriptors while
            # AG-H waits for the previous iteration's tail stage writes.
            PRE_A = 8

            def emit_A(b, p):
                g0, ng = batches[b]
                qA = (2 * b) % NQUEUES
                ch0 = g0 * GROUP_SUBS * A_CHUNKS
                nch = ng * GROUP_SUBS * A_CHUNKS
                nidx = nch * CHUNK
                mA = gpool.tile([128, batch_idx], f16, tag="mA")
                nc.gpsimd.dma_gather(
                    mA[:, :nidx].rearrange("p (c e) -> p c e", e=HID),
                    tabL[p][0:half_rows, :],
                    idxA_sb[:, ch0 * (CHUNK // 16):
                            (ch0 + nch) * (CHUNK // 16)],
                    num_idxs=nidx, num_idxs_reg=nidx,
                    elem_size=HID, single_packet=False, queue_num=qA)
                return mA

            n_hbat = sum(1 for g0, ng in batches if g0 < hgrp)
            for t in range(iters):
                p = t % 2
                last = t == iters - 1
                preA = {b: emit_A(b, p) for b in range(PRE_A)}
                allgather(shardH[p], tabH[p])
                for b, (g0, ng) in enumerate(batches):
                    qB = (2 * b + 1) % NQUEUES
                    ch0 = g0 * GROUP_SUBS * A_CHUNKS        # first A chunk
                    nch = ng * GROUP_SUBS * A_CHUNKS        # chunks (=A=B)
                    nidx = nch * CHUNK
                    mA = preA.pop(b) if b in preA else emit_A(b, p)
                    mB = gpoolB.tile([128, batch_idx], f16, tag="mB")
                    nc.gpsimd.dma_gather(
                        mB[:, :nidx].rearrange("p (c e) -> p c e", e=HID),
                        tabH[p][0:half_rows, :],
                        idxB_sb[:, ch0 * (CHUNK // 16):
                                (ch0 + nch) * (CHUNK // 16)],
                        num_idxs=nidx, num_idxs_reg=nidx,
                        elem_size=HID, single_packet=False, queue_num=qB)
                    for u in range(ng):
                        g = g0 + u
                        gc = slice(g * HID, (g + 1) * HID)
                        ps = ppool.tile([128, HID], f32, tag="ps")
                        for j in range(GROUP_SUBS):
                            sb = g * GROUP_SUBS + j
                            prange = slice(32 * j, 32 * j + 32)
                            tpos = (0, 32 * j)
                            for k in range(A_CHUNKS):
                                cA = sb * A_CHUNKS + k
                                q = (u * GROUP_SUBS + j) * A_CHUNKS + k
                                nc.tensor.matmul(
                                    ps[prange, :],
                                    S_sb[:, cA * 32:(cA + 1) * 32],
                                    mA[:, q * HID:(q + 1) * HID],
                                    start=(k == 0), stop=False,
                                    tile_position=tpos)
                            for k in range(B_CHUNKS):
                                cB = nchA + sb * B_CHUNKS + k
                                q = (u * GROUP_SUBS + j) * B_CHUNKS + k
                                nc.tensor.matmul(
                                    ps[prange, :],
                                    S_sb[:, cB * 32:(cB + 1) * 32],
                                    mB[:, q * HID:(q + 1) * HID],
                                    start=False, stop=(k == B_CHUNKS - 1),
                                    tile_position=tpos)
                        if not last:
                            stage = spool.tile([128, HID], f16, tag="stage")
                            nc.vector.scalar_tensor_tensor(
                                stage[:, :], ps[:, :], dis2g[:, g:g + 1],
                                dxX_sb[:, gc], op0=MULT, op1=ADD)
                            stage_dma(stage, g, (t + 1) % 2)
                        else:
                            u_t = wpool.tile([128, HID], f32, tag="t1")
                            nc.vector.scalar_tensor_tensor(
                                u_t[:, :], ps[:, :], ag[:, g:g + 1],
                                xX_sb[:, gc], op0=MULT, op1=ADD)
                            nc.vector.tensor_scalar_max(u_t[:, :], u_t[:, :],
                                                        0.0)
                            tp_ps = tpool.tile([128, 128], f32, tag="tp")
                            nc.tensor.transpose(tp_ps[:, :], u_t[:, :],
                                                ident[:, :])
                            uT_sb = wpool.tile([128, 128], f32, tag="xT")
                            nc.vector.tensor_copy(uT_sb[:, :], tp_ps[:, :])
                            o_ps = tpool.tile([128, OUT_C], f32, tag="mm2")
                            nc.tensor.matmul(o_ps[:, :], uT_sb[:, :],
                                             Wp_sb[:, :], start=True, stop=True)
                            o_sb = spool.tile([128, OUT_C], f32, tag="osb")
                            nc.vector.tensor_tensor(o_sb[:, :], o_ps[:, :],
                                                    bP_sb[:, :], op=ADD)
                            rows = slice(g * 128, (g + 1) * 128)
                            nc.sync.dma_start(out_h[rows, :], o_sb[:, :])
                    if not last and b == n_hbat + 2:
                        allgather(shardL[(t + 1) % 2], tabL[(t + 1) % 2])


    nc.compile()
    return nc


# ----------------------------------------------------------------------------
# Runner
# ----------------------------------------------------------------------------

def _run(inputs, n=N, ncores=NCORES, trace=False, use_sim=False, iters=POWER1):
    meta, per_core = _preprocess(inputs, n=n, ncores=ncores)
    nc = _build_program(meta, iters=iters)
    in_maps = [dict(pc) for pc in per_core]

    if use_sim:
        from concourse.bass_interp import MultiCoreSim
        sim = MultiCoreSim(nc, num_cores=ncores)
        for c in range(ncores):
            for k, v in in_maps[c].items():
                sim.cores[c].tensor(k)[:] = v
        sim.simulate(check_with_hw=False)
        results = [{"out": np.array(sim.cores[c].tensor("out"))}
                   for c in range(ncores)]
        bres = None
    else:
        from concourse.bass_utils import run_bass_kernel_spmd
        bres = run_bass_kernel_spmd(nc, in_maps, core_ids=list(range(ncores)),
                                    trace=trace)
        results = bres.results

    # unshard: slots -> nodes
    npc, slots = meta["npc"], meta["slots"]
    son = meta["slot_of_node"]
    out = np.zeros((n, OUT_C), dtype=np.float32)
    for c in range(ncores):
        nodes = np.arange(c * npc, (c + 1) * npc)
        out[nodes] = results[c]["out"][son[nodes]]
    return out, bres


def kernel(**inputs) -> np.ndarray:
    # Run twice and compare: guards against rare transient device faults
    # (observed once after an unrecoverable-NRT event on a shared terminal).
    out1, _ = _run(inputs)
    out2, _ = _run(inputs)
    if np.allclose(out1, out2, rtol=0, atol=1e-4):
        return out1
    out3, _ = _run(inputs)
    if np.allclose(out1, out3, rtol=0, atol=1e-4):
        return out1
    return out2 if np.allclose(out2, out3, rtol=0, atol=1e-4) else out3
